# revision 7
# baseline (speedup 1.0000x reference)
"""Conv1d (B=32, C_in=C_out=256, W=4096, K=3, pad=1) on 8 Trainium2 cores.

Strategy: data-parallel over batch (4 per core). Per core the conv is 6
accumulated matmuls per 512-position output chunk: contraction over (tap u,
ci_chunk) with lhsT = weight tile [128 ci x 128 co] and rhs = a padded-x
column block [128 ci x 512+2]. fp16 inputs, fp32 PSUM accumulation, bias
added during the PSUM->SBUF drain on DVE with fp16 output staging (halves
store bytes; host casts back to fp32).

v3 schedule:
- PE warm-up: a memset tile feeds 16 dummy matmuls emitted first, so the
  tensor engine's DVFS ramp (~3.5us of slow matmuls) happens while the
  first x blocks are still in flight.
- batch 0's first four blocks arrive as [128, 258] half tiles (516B rows,
  ~1.7us each) so the first real matmul starts ~2us earlier; chunks 0-3 of
  batch 0 run as 2x6 matmuls of 256 free dim.
- chunk loop is co-interleaved (n outer, co inner) so each x block is
  consumed over ~3us, giving the load rings slack -> no mid-kernel PE
  stalls.
- loads are spread over sync/scalar/gpsimd rings in consumption order;
  stores go on the sync ring after its loads. Outputs are staged per
  (b, co, quarter) [128, 1024] fp16 and flushed when both chunks drain;
  the final quarters are stored per 512-col chunk to keep the tail short.
"""

import numpy as np

F16 = np.float16

B, C, W, K = 32, 256, 4096, 3
NCORES = 8
BPC = B // NCORES          # batches per core
P = 128                    # partitions
CIC = C // P               # ci chunks
COC = C // P               # co chunks
NCH = 512                  # positions per matmul (one PSUM bank of fp32)
NCHUNKS = W // NCH         # position chunks per batch row
BW = NCH + 2               # x block width (512 cols + 2-tap halo)
HW_ = NCH // 2 + 2         # x half-block width (256 cols + halo)
QW = 1024                  # store quarter width
NHALF = 2                  # batch-0 chunks loaded as halves
NWARM = 22                 # PE warm-up matmuls (bridge until first x lands)

_cache = {}


def _build_program():
    import concourse.bass as bass
    import concourse.bacc as bacc
    import concourse.mybir as mybir
    from concourse import tile

    nc = bacc.Bacc(None, target_bir_lowering=False)
    # x, padded by one position on each side, pre-split into NCHUNKS column
    # blocks with a 2-column halo: xb[b, ci, n] covers padded columns
    # n*512 .. n*512+513.
    xb_d = nc.dram_tensor("xb", [BPC, CIC, NCHUNKS, P, BW], mybir.dt.float16,
                          kind="ExternalInput")
    # weight tiles, t = coc*6 + u*CIC + cic, each [ci_in, co_in]
    w_d = nc.dram_tensor("wt", [K * CIC * COC, P, P], mybir.dt.float16,
                         kind="ExternalInput")
    b_d = nc.dram_tensor("bb", [P, COC], mybir.dt.float32,
                         kind="ExternalInput")
    out_d = nc.dram_tensor("out", [BPC, COC, P, W], mybir.dt.float16,
                           kind="ExternalOutput")

    with tile.TileContext(nc) as tc:
        with (
            tc.tile_pool(name="wp", bufs=K * CIC * COC + 2) as wp,
            tc.tile_pool(name="xpool",
                         bufs=BPC * CIC * NCHUNKS + CIC * NHALF) as xpool,
            tc.tile_pool(name="opool", bufs=6) as opool,
            tc.tile_pool(name="pspool", bufs=8, space=bass.MemorySpace.PSUM) as pspool,
        ):
            # -- PE warm-up ----------------------------------------------
            warm = wp.tile([P, NCH], mybir.dt.float16, name="warm", tag="warm")
            nc.gpsimd.memset(warm[:], 0)
            wps = pspool.tile([P, NCH], mybir.dt.float32, name="wps", tag="ps")
            for _ in range(NWARM):
                nc.tensor.matmul(wps[:], warm[:, 0:P], warm[:],
                                 start=True, stop=True)

            # -- tiles ----------------------------------------------------
            x_sb = {}       # full blocks
            xh_sb = {}      # batch-0 half blocks for chunks < NHALF
            for b in range(BPC):
                for ci in range(CIC):
                    for n in range(NCHUNKS):
                        if b == 0 and n < NHALF:
                            for h in range(2):
                                xh_sb[(ci, n, h)] = xpool.tile(
                                    [P, HW_], mybir.dt.float16,
                                    name=f"xh_{ci}_{n}_{h}", tag="xt")
                        else:
                            x_sb[(b, ci, n)] = xpool.tile(
                                [P, BW], mybir.dt.float16,
                                name=f"xt_{b}_{ci}_{n}", tag="xt")
            w_sb = [wp.tile([P, P], mybir.dt.float16, name=f"wt_{t}", tag="wt")
                    for t in range(K * CIC * COC)]
            b_sb = wp.tile([P, COC], mybir.dt.float32)

            # -- load schedule --------------------------------------------
            # batch 0 split by ci chunk across two rings (sync: ci0,
            # gpsimd: ci1) so the supply rate matches the PE's ~3us/block
            # consumption: chunk 0-1 halves first, then full blocks 2-7.
            for ci, ring in ((0, nc.sync), (1, nc.gpsimd)):
                for n in range(NHALF):
                    for h in range(2):
                        ring.dma_start(
                            xh_sb[(ci, n, h)][:],
                            xb_d[0, ci, n][:, h * 256:h * 256 + HW_])
                for n in range(NHALF, NCHUNKS):
                    ring.dma_start(x_sb[(0, ci, n)][:], xb_d[0, ci, n])
            # scalar ring: weights (co0 group first), bias, then batches 2,3
            for t in range(K * CIC * COC):
                nc.scalar.dma_start(w_sb[t][:], w_d[t])
            nc.scalar.dma_start(b_sb[:], b_d[:])
            for b in (2, 3):
                for n in range(NCHUNKS):
                    for ci in range(CIC):
                        nc.scalar.dma_start(x_sb[(b, ci, n)][:], xb_d[b, ci, n])
            # gpsimd ring (after its batch-0 share): batch 1
            for n in range(NCHUNKS):
                for ci in range(CIC):
                    nc.gpsimd.dma_start(x_sb[(1, ci, n)][:], xb_d[1, ci, n])

            # -- compute --------------------------------------------------
            NACC = K * CIC
            o_sb = {}
            for b in range(BPC):
                for n in range(NCHUNKS):
                    q, hq = n // 2, n % 2
                    for co in range(COC):
                        if hq == 0:
                            o_sb[co] = opool.tile(
                                [P, QW], mybir.dt.float16,
                                name=f"ot_{b}_{co}_{q}", tag="ot")
                        ps = pspool.tile([P, NCH], mybir.dt.float32,
                                         name=f"ps_{b}_{co}_{n}", tag="ps")
                        if b == 0 and n < NHALF:
                            for h in range(2):
                                for k, (u, ci) in enumerate(
                                        (u, ci) for u in range(K)
                                        for ci in range(CIC)):
                                    nc.tensor.matmul(
                                        ps[:, h * 256:(h + 1) * 256],
                                        w_sb[co * NACC + u * CIC + ci][:],
                                        xh_sb[(ci, n, h)][:, u:u + 256],
                                        start=(k == 0), stop=(k == NACC - 1),
                                    )
                        else:
                            for k, (u, ci) in enumerate(
                                    (u, ci) for u in range(K)
                                    for ci in range(CIC)):
                                nc.tensor.matmul(
                                    ps[:], w_sb[co * NACC + u * CIC + ci][:],
                                    x_sb[(b, ci, n)][:, u:u + NCH],
                                    start=(k == 0), stop=(k == NACC - 1),
                                )
                        nc.vector.tensor_scalar_add(
                            o_sb[co][:, hq * NCH:(hq + 1) * NCH], ps[:],
                            b_sb[:, co:co + 1],
                        )
                        if b == BPC - 1 and q == 3:
                            # tail: flush the final quarters per chunk
                            nc.sync.dma_start(
                                out_d[b, co, :, n * NCH:(n + 1) * NCH],
                                o_sb[co][:, hq * NCH:(hq + 1) * NCH])
                        elif hq == 1:
                            nc.sync.dma_start(
                                out_d[b, co, :, q * QW:(q + 1) * QW],
                                o_sb[co][:])
    nc.compile()
    return nc


def _prep_inputs(x, weight, bias):
    # x: [32,256,4096] f32 -> padded fp16 blocks [B, CIC, NCHUNKS, 128, 514]
    xp = np.zeros((B, CIC, P, W + 2), F16)
    xp[:, :, :, 1:W + 1] = x.reshape(B, CIC, P, W).astype(F16)
    xb = np.empty((B, CIC, NCHUNKS, P, BW), F16)
    for n in range(NCHUNKS):
        xb[:, :, n] = xp[:, :, :, n * NCH:n * NCH + BW]
    # weight: [co, ci, u] -> tiles [coc*6 + u*CIC + cic][ci_in, co_in]
    wt = weight.reshape(COC, P, CIC, P, K)          # [coc, co_in, cic, ci_in, u]
    w_host = np.ascontiguousarray(
        wt.transpose(0, 4, 2, 3, 1)                 # [coc, u, cic, ci_in, co_in]
    ).reshape(K * CIC * COC, P, P).astype(F16)
    b_host = np.ascontiguousarray(bias.reshape(COC, P).T).astype(np.float32)
    return xb, w_host, b_host


def run(x, weight, bias, trace=False):
    from concourse.bass_utils import run_bass_kernel_spmd

    if "nc" not in _cache:
        _cache["nc"] = _build_program()
    nc = _cache["nc"]

    xb, w_host, b_host = _prep_inputs(
        np.asarray(x, np.float32), np.asarray(weight, np.float32),
        np.asarray(bias, np.float32))
    in_maps = [
        {"xb": xb[c * BPC:(c + 1) * BPC], "wt": w_host, "bb": b_host}
        for c in range(NCORES)
    ]
    res = run_bass_kernel_spmd(nc, in_maps, list(range(NCORES)), trace=trace)
    out = np.concatenate(
        [res.results[c]["out"].reshape(BPC, C, W) for c in range(NCORES)],
        axis=0).astype(np.float32)
    return out, res


def kernel(x, weight, bias):
    out, _ = run(x, weight, bias, trace=False)
    return out


# revision 8
# speedup vs baseline: 1.0360x; 1.0360x over previous
"""Conv1d (B=32, C_in=C_out=256, W=4096, K=3, pad=1) on 8 Trainium2 cores.

Strategy: data-parallel over batch (4 per core). Per core the conv is 6
accumulated matmuls per 512-position output chunk: contraction over (tap u,
ci_chunk) with lhsT = weight tile [128 ci x 128 co] and rhs = a padded-x
column block [128 ci x 512+2]. fp16 inputs, fp32 PSUM accumulation, bias
added during the PSUM->SBUF drain on DVE with fp16 output staging (halves
store bytes; host casts back to fp32).

v3 schedule:
- PE warm-up: a memset tile feeds 16 dummy matmuls emitted first, so the
  tensor engine's DVFS ramp (~3.5us of slow matmuls) happens while the
  first x blocks are still in flight.
- batch 0's first four blocks arrive as [128, 258] half tiles (516B rows,
  ~1.7us each) so the first real matmul starts ~2us earlier; chunks 0-3 of
  batch 0 run as 2x6 matmuls of 256 free dim.
- chunk loop is co-interleaved (n outer, co inner) so each x block is
  consumed over ~3us, giving the load rings slack -> no mid-kernel PE
  stalls.
- loads are spread over sync/scalar/gpsimd rings in consumption order;
  stores go on the sync ring after its loads. Outputs are staged per
  (b, co, quarter) [128, 1024] fp16 and flushed when both chunks drain;
  the final quarters are stored per 512-col chunk to keep the tail short.
"""

import numpy as np

F16 = np.float16

B, C, W, K = 32, 256, 4096, 3
NCORES = 8
BPC = B // NCORES          # batches per core
P = 128                    # partitions
CIC = C // P               # ci chunks
COC = C // P               # co chunks
NCH = 512                  # positions per matmul (one PSUM bank of fp32)
NCHUNKS = W // NCH         # position chunks per batch row
BW = NCH + 2               # x block width (512 cols + 2-tap halo)
HW_ = NCH // 2 + 2         # x half-block width (256 cols + halo)
QW = 1024                  # store quarter width
NHALF = 2                  # batch-0 chunks loaded as halves
NWARM = 10                 # PE warm-up matmuls

_cache = {}


def _build_program():
    import concourse.bass as bass
    import concourse.bacc as bacc
    import concourse.mybir as mybir
    from concourse import tile

    nc = bacc.Bacc(None, target_bir_lowering=False)
    # x, padded by one position on each side, pre-split into NCHUNKS column
    # blocks with a 2-column halo: xb[b, ci, n] covers padded columns
    # n*512 .. n*512+513.
    xb_d = nc.dram_tensor("xb", [BPC, CIC, NCHUNKS, P, BW], mybir.dt.float16,
                          kind="ExternalInput")
    # weight tiles, t = coc*6 + u*CIC + cic, each [ci_in, co_in]
    w_d = nc.dram_tensor("wt", [K * CIC * COC, P, P], mybir.dt.float16,
                         kind="ExternalInput")
    b_d = nc.dram_tensor("bb", [P, COC], mybir.dt.float32,
                         kind="ExternalInput")
    out_d = nc.dram_tensor("out", [BPC, COC, P, W], mybir.dt.float16,
                           kind="ExternalOutput")

    with tile.TileContext(nc) as tc:
        with (
            tc.tile_pool(name="wp", bufs=K * CIC * COC + 2) as wp,
            tc.tile_pool(name="xpool",
                         bufs=BPC * CIC * NCHUNKS + CIC * NHALF) as xpool,
            tc.tile_pool(name="opool", bufs=6) as opool,
            tc.tile_pool(name="pspool", bufs=8, space=bass.MemorySpace.PSUM) as pspool,
        ):
            # -- PE warm-up ----------------------------------------------
            warm = wp.tile([P, NCH], mybir.dt.float16, name="warm", tag="warm")
            nc.gpsimd.memset(warm[:], 0)
            wps = pspool.tile([P, NCH], mybir.dt.float32, name="wps", tag="ps")
            for _ in range(NWARM):
                nc.tensor.matmul(wps[:], warm[:, 0:P], warm[:],
                                 start=True, stop=True)

            # -- tiles ----------------------------------------------------
            x_sb = {}       # full blocks
            xh_sb = {}      # batch-0 half blocks for chunks < NHALF
            for b in range(BPC):
                for ci in range(CIC):
                    for n in range(NCHUNKS):
                        if b == 0 and n < NHALF:
                            for h in range(2):
                                xh_sb[(ci, n, h)] = xpool.tile(
                                    [P, HW_], mybir.dt.float16,
                                    name=f"xh_{ci}_{n}_{h}", tag="xt")
                        else:
                            x_sb[(b, ci, n)] = xpool.tile(
                                [P, BW], mybir.dt.float16,
                                name=f"xt_{b}_{ci}_{n}", tag="xt")
            w_sb = [wp.tile([P, P], mybir.dt.float16, name=f"wt_{t}", tag="wt")
                    for t in range(K * CIC * COC)]
            b_sb = wp.tile([P, COC], mybir.dt.float32)

            # -- load schedule --------------------------------------------
            # batch 0 split by ci chunk across two rings (sync: ci0,
            # gpsimd: ci1) so the supply rate matches the PE's ~3us/block
            # consumption: chunk 0-1 halves first, then full blocks 2-7.
            for ci, ring in ((0, nc.sync), (1, nc.gpsimd)):
                for n in range(NHALF):
                    for h in range(2):
                        ring.dma_start(
                            xh_sb[(ci, n, h)][:],
                            xb_d[0, ci, n][:, h * 256:h * 256 + HW_])
                for n in range(NHALF, NCHUNKS):
                    ring.dma_start(x_sb[(0, ci, n)][:], xb_d[0, ci, n])
            # scalar ring: weights (co0 group first), bias, then batches 2,3
            for t in range(K * CIC * COC):
                nc.scalar.dma_start(w_sb[t][:], w_d[t])
            nc.scalar.dma_start(b_sb[:], b_d[:])
            for b in (2, 3):
                for n in range(NCHUNKS):
                    for ci in range(CIC):
                        nc.scalar.dma_start(x_sb[(b, ci, n)][:], xb_d[b, ci, n])
            # gpsimd ring (after its batch-0 share): batch 1
            for n in range(NCHUNKS):
                for ci in range(CIC):
                    nc.gpsimd.dma_start(x_sb[(1, ci, n)][:], xb_d[1, ci, n])

            # -- compute --------------------------------------------------
            NACC = K * CIC
            o_sb = {}
            for b in range(BPC):
                for n in range(NCHUNKS):
                    q, hq = n // 2, n % 2
                    for co in range(COC):
                        if hq == 0:
                            o_sb[co] = opool.tile(
                                [P, QW], mybir.dt.float16,
                                name=f"ot_{b}_{co}_{q}", tag="ot")
                        ps = pspool.tile([P, NCH], mybir.dt.float32,
                                         name=f"ps_{b}_{co}_{n}", tag="ps")
                        if b == 0 and n < NHALF:
                            for h in range(2):
                                for k, (u, ci) in enumerate(
                                        (u, ci) for u in range(K)
                                        for ci in range(CIC)):
                                    nc.tensor.matmul(
                                        ps[:, h * 256:(h + 1) * 256],
                                        w_sb[co * NACC + u * CIC + ci][:],
                                        xh_sb[(ci, n, h)][:, u:u + 256],
                                        start=(k == 0), stop=(k == NACC - 1),
                                    )
                        else:
                            for k, (u, ci) in enumerate(
                                    (u, ci) for u in range(K)
                                    for ci in range(CIC)):
                                nc.tensor.matmul(
                                    ps[:], w_sb[co * NACC + u * CIC + ci][:],
                                    x_sb[(b, ci, n)][:, u:u + NCH],
                                    start=(k == 0), stop=(k == NACC - 1),
                                )
                        nc.vector.tensor_scalar_add(
                            o_sb[co][:, hq * NCH:(hq + 1) * NCH], ps[:],
                            b_sb[:, co:co + 1],
                        )
                        if b == BPC - 1 and q == 3:
                            # tail: flush the final quarters per chunk
                            nc.sync.dma_start(
                                out_d[b, co, :, n * NCH:(n + 1) * NCH],
                                o_sb[co][:, hq * NCH:(hq + 1) * NCH])
                        elif hq == 1:
                            nc.sync.dma_start(
                                out_d[b, co, :, q * QW:(q + 1) * QW],
                                o_sb[co][:])
    nc.compile()
    return nc


def _prep_inputs(x, weight, bias):
    # x: [32,256,4096] f32 -> padded fp16 blocks [B, CIC, NCHUNKS, 128, 514]
    xp = np.zeros((B, CIC, P, W + 2), F16)
    xp[:, :, :, 1:W + 1] = x.reshape(B, CIC, P, W).astype(F16)
    xb = np.empty((B, CIC, NCHUNKS, P, BW), F16)
    for n in range(NCHUNKS):
        xb[:, :, n] = xp[:, :, :, n * NCH:n * NCH + BW]
    # weight: [co, ci, u] -> tiles [coc*6 + u*CIC + cic][ci_in, co_in]
    wt = weight.reshape(COC, P, CIC, P, K)          # [coc, co_in, cic, ci_in, u]
    w_host = np.ascontiguousarray(
        wt.transpose(0, 4, 2, 3, 1)                 # [coc, u, cic, ci_in, co_in]
    ).reshape(K * CIC * COC, P, P).astype(F16)
    b_host = np.ascontiguousarray(bias.reshape(COC, P).T).astype(np.float32)
    return xb, w_host, b_host


def run(x, weight, bias, trace=False):
    from concourse.bass_utils import run_bass_kernel_spmd

    if "nc" not in _cache:
        _cache["nc"] = _build_program()
    nc = _cache["nc"]

    xb, w_host, b_host = _prep_inputs(
        np.asarray(x, np.float32), np.asarray(weight, np.float32),
        np.asarray(bias, np.float32))
    in_maps = [
        {"xb": xb[c * BPC:(c + 1) * BPC], "wt": w_host, "bb": b_host}
        for c in range(NCORES)
    ]
    res = run_bass_kernel_spmd(nc, in_maps, list(range(NCORES)), trace=trace)
    out = np.concatenate(
        [res.results[c]["out"].reshape(BPC, C, W) for c in range(NCORES)],
        axis=0).astype(np.float32)
    return out, res


def kernel(x, weight, bias):
    out, _ = run(x, weight, bias, trace=False)
    return out
